# revision 24
# baseline (speedup 1.0000x reference)
"""Causal self-attention (B=2, T=2048, dim=2048, 16 heads, RoPE) on 8 trn2
NeuronCores.

Sharding: core c handles batch b = c//4 and head group g = c%4 (4 heads each,
tensor-parallel over heads). Each core computes QKV projection + RoPE +
causal attention + its partial out-projection; the host sums the 4 partial
out-proj results per batch (the "all-reduce") and stacks batches.

Design (evolved 566us -> 405us phase-split -> 368us fused):
  - All matmuls bf16; QKV/attention/out-proj fully software-pipelined so the
    PE never idles long enough for the HAM clock-gate to re-throttle (the
    phase-split 405us version lost ~45us to start/transition/drain stalls
    plus the cold-clock stretch they caused).
  - Region R0: t-slice 0 QKV, kc-outer 6-psum groups (tracks weight-chunk
    DMA arrival).  Regions R1..R3: t-slice t QKV restructured output-outer
    (one psum at a time, bufs=2) woven slot-by-slot with super-block t-1's
    attention gpairs; sb's normalize chain rides the remaining slots.
    POST: sb3's attention woven with sb0-2's out-proj units, then sb3's
    units drain behind a PE-broadcast normalize.
  - PSUM budget: R0: 8-bank QKV ring. R1-3: qkv(2) + S^T(2x2) + O(1) +
    sums(1) = 8. POST: S^T(4) + O(1) + sums(1) + out-proj(2) = 8.
  - DMA: the critical first loads (x t-slice 0 jj-blocks + w_qkv halves,
    hp1 before hp0) are byte-balanced round-robin across the Sync and
    Scalar HWDGE queues in need order (first matmul ~10us instead of
    ~27us; R0's kc cadence needs ~300GB/s of weights, beyond one queue);
    x per-jj tiles so the first matmul waits on exactly one DMA; later x
    slices spread across all three queues; w_out has its own SBUF (the
    405us version's w_out DMA was WAR-blocked behind all of phase A and
    landed at ~205us, stalling the first out-proj units).  x tiles ring
    bufs=2 (tsl2/3 reuse tsl0/1's buffers).
  - RoPE rotate-half is a partition-shifted SBUF->SBUF DMA copy (sign
    folded into the host sin table); evac intermediates bf16 (halves DVE
    and rot-DMA cost vs f32).
  - Softmax sums: DVE pair-reduces each gpair's two masked exp chunks
    (and pairs consecutive non-diagonal gpairs again) so the one-hot PE
    sum matmuls stream each key only once per 2-4 chunks into a shared
    [4, 512] PSUM tile, emitted two entries late; one [4,512] bf16
    reciprocal; per-head normalize is broadcast via GpSimd (sb0-2,
    multiplied in place into the otu tile) or PE outer product (sb3
    drain).
  - Diagonal key-chunks stream only their valid query range (causal trim).
  - QKV bias via Scalar-engine activation bias; out-bias added on host;
    y written bf16 and upcast on host.
"""

import math
import os
import sys
import types

import numpy as np
import ml_dtypes

BF16NP = ml_dtypes.bfloat16

# ---------------------------------------------------------------------------
# NTFF profile hook (missing antenv.axon_hooks in this image). Reconstructed
# so run_bass_kernel_spmd(trace=True) can measure HW exec time.
# ---------------------------------------------------------------------------
try:
    import antenv

    if "antenv.axon_hooks" not in sys.modules:
        try:
            from trn_agent_boot.trn_boot import _ntff_profile_via_ctypes

            _hook = _ntff_profile_via_ctypes("/opt/axon/libaxon_pjrt.so")
        except Exception:
            _hook = None
        _m = types.ModuleType("antenv.axon_hooks")
        _m.get_axon_ntff_profile_hook = lambda: _hook
        _m.set_axon_ntff_profile_hook = lambda h: None
        sys.modules["antenv.axon_hooks"] = _m
        antenv.axon_hooks = _m
except Exception:
    pass

import concourse.bass as bass
import concourse.tile as tile
from concourse import bacc, mybir
from concourse.bass_utils import run_bass_kernel_spmd

# Problem constants (hardcoded per the task contract).
B = 2
T = 2048
DIM = 2048
H = 16
HD = 128                  # head_dim
G = 4                     # head groups (cores per batch)
HPG = H // G              # heads per group = 4
N_CORES = 8
SCALE = 1.0 / math.sqrt(HD)

F32 = mybir.dt.float32
BF16 = mybir.dt.bfloat16

TSL = 512                 # t-slice width in the projection phase
NTSL = T // TSL           # 4
QSB = 512                 # query super-block width in the attention phase
NSB = T // QSB            # 4
KC = 128                  # key chunk (partition dim)
HEAD_ORDER = (2, 3, 0, 1)

LAST_EXEC_NS = None
LAST_RESULTS = None

_PROGRAM_CACHE = {}


def _build_program():
    nc = bacc.Bacc("TRN2", target_bir_lowering=False, debug=False,
                   num_devices=N_CORES)

    xT = nc.dram_tensor("xT", [DIM, T], BF16, kind="ExternalInput").ap()
    w_qkv = nc.dram_tensor("w_qkv_loc", [DIM, 3 * HPG * HD], BF16,
                           kind="ExternalInput").ap()
    b_cols = nc.dram_tensor("b_cols", [HD, 3 * HPG], F32,
                            kind="ExternalInput").ap()
    w_out = nc.dram_tensor("w_out_loc", [HPG * HD, DIM], BF16,
                           kind="ExternalInput").ap()
    cosT = nc.dram_tensor("cosT", [HD, T], BF16, kind="ExternalInput").ap()
    sinT = nc.dram_tensor("sinTs", [HD, T], BF16, kind="ExternalInput").ap()
    masks = nc.dram_tensor("masks_t", [KC, KC], BF16,
                           kind="ExternalInput").ap()
    y = nc.dram_tensor("y_part", [T, DIM], BF16, kind="ExternalOutput").ap()

    with tile.TileContext(nc) as tc:
        _emit(tc, nc, xT, w_qkv, b_cols, w_out, cosT, sinT, masks, y)

    nc.compile()
    return nc


def _emit(tc, nc, xT, w_qkv, b_cols_d, w_out, cosT_d, sinT_d, masks_d, y):
    from contextlib import ExitStack

    ctx = ExitStack()
    with ctx:
        ctx.enter_context(nc.allow_low_precision(
            reason="bf16 matmul operands and elementwise pipeline"))
        # ---------------- constants (live for the whole kernel) -----------
        consts = ctx.enter_context(tc.tile_pool(name="consts", bufs=1))
        bcols = consts.tile([HD, 3 * HPG], F32, tag="bcols")
        nc.gpsimd.dma_start(out=bcols, in_=b_cols_d)
        # ones4[:, h, :] is the [128, 4] one-hot stationary for head h: only
        # column h is ones, so head h's softmax-sum matmul lands in row h of
        # the shared [HPG, QSB] PSUM accumulator (other rows accumulate +0).
        ones4 = consts.tile([KC, HPG, HPG], BF16, tag="ones4")
        nc.vector.memset(ones4, 0.0)
        for h in range(HPG):
            nc.vector.memset(ones4[:, h, h:h + 1], 1.0)
        ones_row = consts.tile([1, KC], BF16, tag="ones_row")
        nc.vector.memset(ones_row, 1.0)

        # QKV results: SBUF-resident for the whole kernel, split per
        # (head, t-slice) so each attention read depends on exactly the
        # producing t-slice.
        qkv_pool = ctx.enter_context(tc.tile_pool(name="qkv", bufs=1))
        qtr = [[qkv_pool.tile([HD, TSL], BF16, tag=f"qtr{h}_{t}",
                              name=f"qtr{h}_{t}") for t in range(NTSL)]
               for h in range(HPG)]
        ktr = [[qkv_pool.tile([HD, TSL], BF16, tag=f"ktr{h}_{t}",
                              name=f"ktr{h}_{t}") for t in range(NTSL)]
               for h in range(HPG)]
        vh = [[qkv_pool.tile([KC, TSL // KC, HD], BF16, tag=f"vh{h}_{t}",
                             name=f"vh{h}_{t}") for t in range(NTSL)]
              for h in range(HPG)]

        rope = ctx.enter_context(tc.tile_pool(name="rope", bufs=1))
        cosT = rope.tile([HD, T], BF16, tag="cosT")
        sinT = rope.tile([HD, T], BF16, tag="sinT")

        xT_r = xT.rearrange("(c p) t -> p c t", p=KC)        # [128, 16, T]
        w_r = w_qkv.rearrange("(c p) f -> p c f", p=KC)      # [128, 16, 1536]
        NKCH = DIM // KC                                     # 16 k-chunks
        w_out_r = w_out.rearrange("(c p) o -> p c o", p=KC)

        # ------------------- bulk-load issue plan --------------------------
        # Sync (SP) HWDGE queue: the four jj-blocks of x t-slice 0 (the
        # first matmul waits on exactly the first of these), then the
        # latency-critical small DMAs emitted throughout (rot copies, V
        # transposes, r1 stages, y writes).
        # Scalar HWDGE queue: all 16 w_qkv k-chunk rows (full 1536-col
        # rows; R0 consumes them kc-by-kc at ~1.3us each so the queue
        # stays ahead), then x slices 1..3.
        # GpSimd SWDGE queue: bias cols, rope tables, masks, w_out.
        xs_pa = ctx.enter_context(tc.tile_pool(name="xs_a", bufs=3))
        xs_pb = ctx.enter_context(tc.tile_pool(name="xs_b", bufs=2))

        def load_xs(tsl, qs):
            t0 = tsl * TSL
            tiles = []
            for jj in range(4):
                pool = xs_pa if jj in (0, 3) else xs_pb
                xs = pool.tile([KC, 4, TSL], BF16, tag=f"xs{jj}",
                               name=f"xs{tsl}_{jj}")
                qs[jj % len(qs)].dma_start(
                    out=xs, in_=xT_r[:, jj * 4:(jj + 1) * 4, t0:t0 + TSL])
                tiles.append(xs)
            return tiles

        xs = [None] * NTSL
        xs[0] = [(xs_pa if jj in (0, 3) else xs_pb).tile(
                     [KC, 4, TSL], BF16, tag=f"xs{jj}", name=f"xs0_{jj}")
                 for jj in range(4)]

        a_w = ctx.enter_context(tc.tile_pool(name="a_w", bufs=1))
        w_all = a_w.tile([KC, NKCH, 3 * HPG * HD], BF16, tag="w_all")
        # R0 consumes the hp1 (heads 2,3) half of each w row at ~150 GB/s
        # and x jj-blocks at ~100 GB/s -- more than one HWDGE queue
        # sustains.  Issue the critical loads in need order, round-robin
        # across the Sync and Scalar queues; the hp0 halves follow (needed
        # one hp-pass later), then x t-slice 1.
        crit = [("x", 0)]
        for kc in range(NKCH):
            if kc % 4 == 0 and kc > 0:
                crit.append(("x", kc // 4))
            crit.append(("w", (kc, 1)))
        crit += [("w", (kc, 0)) for kc in range(NKCH)]
        qbytes = [0, 0]
        for kind, arg in crit:
            qi = 0 if qbytes[0] <= qbytes[1] else 1
            q = (nc.sync, nc.scalar)[qi]
            if kind == "x":
                jj = arg
                q.dma_start(out=xs[0][jj],
                            in_=xT_r[:, jj * 4:(jj + 1) * 4, 0:TSL])
                qbytes[qi] += 4 * TSL * KC * 2
            else:
                kc, hp = arg
                q.dma_start(out=w_all[:, kc, hp * 768:hp * 768 + 768],
                            in_=w_r[:, kc, hp * 768:hp * 768 + 768])
                qbytes[qi] += 768 * KC * 2
        xs[1] = load_xs(1, [nc.sync, nc.scalar])

        # GpSimd (SWDGE) carries the relaxed loads: rope tables (needed at
        # ~25us), the hp0 w halves (R0's second hp pass, from ~33us), the
        # diagonal masks, then w_out.
        nc.gpsimd.dma_start(out=cosT, in_=cosT_d)
        nc.gpsimd.dma_start(out=sinT, in_=sinT_d)
        # masks_sb[r, c] = (c >= r).  A diagonal chunk dj only masks its
        # first 128 columns (beyond that c - dj*128 >= 128 > r always), so
        # one [128, 128] triangular corner serves every dj.
        masks_sb = consts.tile([KC, KC], BF16, tag="masks")
        nc.gpsimd.dma_start(out=masks_sb, in_=masks_d)
        c_w = ctx.enter_context(tc.tile_pool(name="c_w", bufs=1))
        wo = c_w.tile([KC, HPG, DIM], BF16, tag="wo")
        for hc in range(HPG):
            nc.gpsimd.dma_start(out=wo[:, hc, :], in_=w_out_r[:, hc, :])

        # ------------------- shared evac / attention pools -----------------
        a_vb = ctx.enter_context(tc.tile_pool(name="a_vb", bufs=2))
        a_qb = ctx.enter_context(tc.tile_pool(name="a_qb", bufs=4))
        a_rot = ctx.enter_context(tc.tile_pool(name="a_rot", bufs=4))
        a_m1 = ctx.enter_context(tc.tile_pool(name="a_m1", bufs=4))
        b_pt = ctx.enter_context(tc.tile_pool(name="b_pt", bufs=2))
        b_tm = ctx.enter_context(tc.tile_pool(name="b_tm", bufs=4))
        b_otn = ctx.enter_context(tc.tile_pool(name="b_otn", bufs=1))
        b_sm = ctx.enter_context(tc.tile_pool(name="b_sm", bufs=1))
        b_rb = ctx.enter_context(tc.tile_pool(name="b_rb", bufs=2))
        c_sb = ctx.enter_context(tc.tile_pool(name="c_sb", bufs=3))

        def feat0(h, kind):
            # w_qkv_loc is host-packed head-pair-major:
            # [hp0: q(2 heads), k, v | hp1: q, k, v], 256 cols per block.
            return (h // 2) * 768 + kind * 256 + (h % 2) * HD

        # ---------------- per-head QKV evacuation (R1..R3) ----------------
        def evac_v(t, h, ps_v):
            vb = a_vb.tile([HD, TSL], BF16, tag="vb")
            nc.vector.tensor_scalar_add(
                vb, ps_v, bcols[:, 2 * HPG + h:2 * HPG + h + 1])
            nc.sync.dma_start_transpose(out=vh[h][t], in_=vb)

        def evac_qk_start(h, kind, ps):
            # psum -> bf16 with bias, then launch the rotate-half copies
            # (partition-shifted SBUF->SBUF DMA; sign folded into sin).
            qb = a_qb.tile([HD, TSL], BF16, tag="qb")
            nc.scalar.activation(
                qb, ps, mybir.ActivationFunctionType.Identity,
                bias=bcols[:, kind * HPG + h:kind * HPG + h + 1])
            half = HD // 2
            qrot = a_rot.tile([HD, TSL], BF16, tag="qrot")
            nc.sync.dma_start(out=qrot[0:half, :], in_=qb[half:HD, :])
            nc.sync.dma_start(out=qrot[half:HD, :], in_=qb[0:half, :])
            return qb, qrot

        def rope_m1(t, qb):
            t0 = t * TSL
            m1 = a_m1.tile([HD, TSL], BF16, tag="m1")
            nc.vector.tensor_mul(m1, qb, cosT[:, t0:t0 + TSL])
            return m1

        def rope_fin(t, h, kind, qrot, m1):
            # m2 in place on qrot, then add into the destination tile.
            t0 = t * TSL
            nc.vector.tensor_mul(qrot, qrot, sinT[:, t0:t0 + TSL])
            dst = qtr[h] if kind == 0 else ktr[h]
            nc.vector.tensor_add(dst[t], m1, qrot)

        # ---------------- attention pop-list builder -----------------------
        # Each pop emits ~one gpair of PE work; the QKV weave calls them at
        # regular slots so emission order (= engine queue order) interleaves.
        otu_s = [dict() for _ in range(NSB)]
        otn_s = [dict() for _ in range(NSB)]
        r1_s = [dict() for _ in range(NSB)]

        def build_attn_pops(csb, ps_st_pool, ps_o_pool, ps_sum_pool):
            nk = (csb + 1) * (QSB // KC)
            ngp = nk // 2
            st = {"sum_first": True, "q": [], "ps_sum": None, "ps_o": None,
                  "tm_prev": None}

            def flush_sum(last):
                while st["q"] and (len(st["q"]) > 2 or last):
                    fh, ftm, fc0 = st["q"].pop(0)
                    nc.tensor.matmul(
                        st["ps_sum"][:, fc0:], ones4[:, fh, :], ftm[:, fc0:],
                        start=st["sum_first"],
                        stop=(last and not st["q"]),
                    )
                    st["sum_first"] = False

            pops = []

            def mk_gpair(h, gpair):
                def emit():
                    if st["ps_sum"] is None:
                        st["ps_sum"] = ps_sum_pool.tile(
                            [HPG, QSB], F32, tag="ps_sum",
                            name=f"ps_sum{csb}")
                    if gpair == 0:
                        st["ps_o"] = ps_o_pool.tile(
                            [HD, QSB], F32, tag="ps_o",
                            name=f"ps_o{csb}_{h}")
                    ps_o = st["ps_o"]
                    k0 = 2 * gpair
                    # Diagonal chunks (dj >= 0) only attend to queries
                    # q >= dj*128: trim the streamed column range of the
                    # S/O/sum matmuls, exp, and mask to the valid part.
                    djs = [(k0 + j) - (nk - QSB // KC) for j in range(2)]
                    c0s = [max(dj, 0) * KC for dj in djs]
                    cmin = min(c0s)
                    ps_st = ps_st_pool.tile([KC, 2, QSB], F32, tag="ps_st",
                                            name=f"ps_st{csb}_{h}_{gpair}")
                    for j in range(2):
                        c0 = c0s[j]
                        kci = k0 + j
                        nc.tensor.matmul(
                            ps_st[:, j, c0:],
                            ktr[h][kci // (TSL // KC)]
                               [:, (kci % (TSL // KC)) * KC:
                                (kci % (TSL // KC) + 1) * KC],
                            qtr[h][csb][:, c0:],
                            start=True, stop=True,
                        )
                    pt = b_pt.tile([KC, 2, QSB], BF16, tag="pt",
                                   name=f"pt{csb}_{h}_{gpair}")
                    nc.scalar.activation(
                        pt[:, :, cmin:], ps_st[:, :, cmin:],
                        mybir.ActivationFunctionType.Exp, scale=SCALE)
                    for j in range(2):
                        dj = djs[j]
                        if dj >= 0:
                            c0 = c0s[j]
                            nc.vector.tensor_mul(
                                pt[:, j, c0:c0 + KC], pt[:, j, c0:c0 + KC],
                                masks_sb)
                    for j in range(2):
                        kci = k0 + j
                        nc.tensor.matmul(
                            ps_o[:, c0s[j]:],
                            vh[h][kci // (TSL // KC)]
                              [:, kci % (TSL // KC), :],
                            pt[:, j, c0s[j]:],
                            start=(kci == 0), stop=(kci == nk - 1),
                        )
                    # Pair-reduce the two (masked) chunks on DVE so the
                    # softmax-sum matmul streams each gpair once instead of
                    # twice.  Diagonal gpairs: chunk 1's [cmin, c1) region
                    # holds exp() of stale psum -- zero it first.
                    if djs[1] >= 0 and c0s[1] > cmin:
                        nc.vector.memset(pt[:, 1, cmin:c0s[1]], 0.0)
                    tm = b_tm.tile([KC, QSB], BF16, tag="tm",
                                   name=f"tm{csb}_{h}_{gpair}")
                    nc.vector.tensor_add(tm[:, cmin:], pt[:, 0, cmin:],
                                         pt[:, 1, cmin:])
                    if djs[1] < 0:
                        # non-diagonal (full-range): pair with the head's
                        # previous non-diag gpair before the PE sum
                        if st["tm_prev"] is None:
                            st["tm_prev"] = tm
                        else:
                            nc.vector.tensor_add(tm, tm, st["tm_prev"])
                            st["tm_prev"] = None
                            st["q"].append((h, tm, 0))
                    else:
                        st["q"].append((h, tm, cmin))
                    flush_sum(False)
                return emit

            def mk_end_head(h):
                def emit():
                    ou = b_otn.tile([HD, QSB], BF16, tag=f"o_{csb}_{h}",
                                    name=f"otu{csb}_{h}")
                    nc.vector.tensor_copy(ou, st["ps_o"])
                    otu_s[csb][h] = ou
                return emit

            for h in HEAD_ORDER:
                for gpair in range(ngp):
                    pops.append(mk_gpair(h, gpair))
                pops.append(mk_end_head(h))

            def fin_sums():
                flush_sum(True)
                rs = b_sm.tile([HPG, QSB], BF16, tag="rsums",
                               name=f"rsums{csb}")
                nc.vector.reciprocal(rs, st["ps_sum"])
                r1_s[csb][0] = rs[0:1, :]
                for h in HEAD_ORDER:
                    if h == 0:
                        continue
                    r1 = b_sm.tile([1, QSB], BF16, tag=f"r1_{h}",
                                   name=f"r1_{csb}_{h}")
                    nc.gpsimd.dma_start(out=r1, in_=rs[h:h + 1, :])
                    r1_s[csb][h] = r1
            pops.append(fin_sums)

            if csb < NSB - 1:
                def mk_norm(h):
                    def emit():
                        rb = b_rb.tile([KC, QSB], BF16, tag="rb",
                                       name=f"rb{csb}_{h}")
                        nc.gpsimd.partition_broadcast(
                            rb, r1_s[csb][h], channels=KC)
                        ou = otu_s[csb][h]
                        nc.vector.tensor_mul(ou, ou, rb)
                        otn_s[csb][h] = ou
                    return emit
                for h in HEAD_ORDER:
                    pops.append(mk_norm(h))
            return pops

        # =================== R0: t-slice 0 QKV (kc-outer) ==================
        # Six psum accumulators advance together so the PE tracks
        # weight-chunk DMA arrival instead of stalling on the full load.
        # Heads (2,3) first: region R1's first attention pops are head 2.
        with tc.tile_pool(name="a_ps0", bufs=8, space="PSUM") as a_ps0:
            for hp in (1, 0):
                heads = (2 * hp, 2 * hp + 1)
                outs = [(h, kind) for h in heads for kind in range(3)]
                pstiles = {}
                for (h, kind) in outs:
                    pstiles[(h, kind)] = a_ps0.tile(
                        [HD, TSL], F32, tag="ps_qkv",
                        name=f"ps0_{h}_{kind}")
                for kc in range(NKCH):
                    for (h, kind) in outs:
                        nc.tensor.matmul(
                            pstiles[(h, kind)],
                            w_all[:, kc, feat0(h, kind):
                                  feat0(h, kind) + HD],
                            xs[0][kc // 4][:, kc % 4, :],
                            start=(kc == 0), stop=(kc == NKCH - 1),
                        )
                # Staged evacuation (stage fans across its engine queue
                # before the next depends on it): V evacs -> q/k evacs ->
                # transposes -> rotations -> m1 -> m2/add.
                vbs, qbs, qrots, m1s = {}, {}, {}, {}
                for h in heads:
                    vb = a_vb.tile([HD, TSL], BF16, tag="vb")
                    nc.vector.tensor_scalar_add(
                        vb, pstiles[(h, 2)],
                        bcols[:, 2 * HPG + h:2 * HPG + h + 1])
                    vbs[h] = vb
                for h in heads:
                    for kind in (0, 1):
                        qb = a_qb.tile([HD, TSL], BF16, tag="qb")
                        if hp == 0 and kind == 1:
                            nc.vector.tensor_scalar_add(
                                qb, pstiles[(h, kind)],
                                bcols[:, kind * HPG + h:
                                      kind * HPG + h + 1])
                        else:
                            nc.scalar.activation(
                                qb, pstiles[(h, kind)],
                                mybir.ActivationFunctionType.Identity,
                                bias=bcols[:, kind * HPG + h:
                                           kind * HPG + h + 1])
                        qbs[(h, kind)] = qb
                for h in heads:
                    nc.sync.dma_start_transpose(out=vh[h][0], in_=vbs[h])
                half = HD // 2
                for h in heads:
                    for kind in (0, 1):
                        qb = qbs[(h, kind)]
                        qrot = a_rot.tile([HD, TSL], BF16, tag="qrot")
                        nc.sync.dma_start(out=qrot[0:half, :],
                                          in_=qb[half:HD, :])
                        nc.sync.dma_start(out=qrot[half:HD, :],
                                          in_=qb[0:half, :])
                        qrots[(h, kind)] = qrot
                for h in heads:
                    for kind in (0, 1):
                        m1s[(h, kind)] = rope_m1(0, qbs[(h, kind)])
                for h in heads:
                    for kind in (0, 1):
                        rope_fin(0, h, kind, qrots[(h, kind)],
                                 m1s[(h, kind)])

        xs[2] = load_xs(2, [nc.gpsimd, nc.sync, nc.scalar, nc.gpsimd])

        # ============ R1..R3: QKV (output-outer) + attention weave =========
        with (
            tc.tile_pool(name="b_ps_s", bufs=2, space="PSUM") as b_ps_s,
            tc.tile_pool(name="b_ps_o", bufs=1, space="PSUM") as b_ps_o,
            tc.tile_pool(name="b_ps_sum", bufs=1, space="PSUM") as b_ps_sum,
        ):
            with tc.tile_pool(name="a_ps", bufs=2, space="PSUM") as a_ps:
                for t in range(1, NTSL):
                    pops = build_attn_pops(t - 1, b_ps_s, b_ps_o, b_ps_sum)
                    npop = len(pops)
                    nslot = 4 * 3 * 4
                    popped = 0
                    sl = 0

                    def slot():
                        nonlocal popped, sl
                        sl += 1
                        target = (npop * sl) // nslot
                        while popped < target:
                            pops[popped]()
                            popped += 1

                    pending_rope = []
                    for h in HEAD_ORDER:
                        # v output first: its transpose DMA overlaps the
                        # q/k matmuls; rot DMAs get a full output block in
                        # flight before m2 enters the DVE queue.
                        psd = {}
                        for kind in (2, 0, 1):
                            ps = a_ps.tile([HD, TSL], F32, tag="ps_qkv",
                                           name=f"ps{t}_{h}_{kind}")
                            for kc in range(NKCH):
                                nc.tensor.matmul(
                                    ps, w_all[:, kc, feat0(h, kind):
                                              feat0(h, kind) + HD],
                                    xs[t][kc // 4][:, kc % 4, :],
                                    start=(kc == 0), stop=(kc == NKCH - 1))
                                if kc % 4 == 3 and kc != NKCH - 1:
                                    slot()
                            psd[kind] = ps
                            if kind == 2:
                                evac_v(t, h, ps)
                                for cl in pending_rope:
                                    cl()
                                pending_rope = []
                            elif kind == 0:
                                psd["qb0"], psd["qr0"] = \
                                    evac_qk_start(h, 0, ps)
                            else:
                                qb1, qr1 = evac_qk_start(h, 1, ps)
                                m10 = rope_m1(t, psd["qb0"])
                                m11 = rope_m1(t, qb1)

                                def fin(t=t, h=h, qr0=psd["qr0"],
                                        qr1=qr1, m10=m10, m11=m11):
                                    rope_fin(t, h, 0, qr0, m10)
                                    rope_fin(t, h, 1, qr1, m11)
                                pending_rope = [fin]
                            slot()
                    for cl in pending_rope:
                        cl()
                    while popped < npop:
                        pops[popped]()
                        popped += 1
                    if t == 1:
                        xs[3] = load_xs(
                            3, [nc.gpsimd, nc.sync, nc.scalar,
                                nc.gpsimd])

            # ================= POST: sb3 attention + out-proj ==============
            NOB = DIM // 512
            NU = (QSB // KC) * NOB                # 16 units per super-block
            with tc.tile_pool(name="c_ps", bufs=2, space="PSUM") as c_ps:
                def emit_c_unit(csb, u):
                    tb, ob = divmod(u, NOB)
                    tt0 = tb * KC
                    o0 = ob * 512
                    ps_y = c_ps.tile([KC, 512], F32, tag="ps_y")
                    for i, hc in enumerate(HEAD_ORDER):
                        nc.tensor.matmul(
                            ps_y, otn_s[csb][hc][:, tt0:tt0 + KC],
                            wo[:, hc, o0:o0 + 512],
                            start=(i == 0), stop=(i == HPG - 1),
                        )
                    ys = c_sb.tile([KC, 512], BF16, tag="ys")
                    r0 = csb * QSB + tt0
                    if u % 2 == 0:
                        nc.scalar.activation(
                            ys, ps_y, mybir.ActivationFunctionType.Identity)
                        nc.scalar.dma_start(
                            out=y[r0:r0 + KC, o0:o0 + 512], in_=ys)
                    else:
                        nc.vector.tensor_copy(ys, ps_y)
                        nc.sync.dma_start(
                            out=y[r0:r0 + KC, o0:o0 + 512], in_=ys)

                pops = build_attn_pops(3, b_ps_s, b_ps_o, b_ps_sum)
                units = [(csb, u) for csb in range(NSB - 1)
                         for u in range(NU)]
                ue = 0
                for i, p in enumerate(pops):
                    p()
                    utarget = (len(units) * (i + 1)) // len(pops)
                    while ue < utarget:
                        emit_c_unit(*units[ue])
                        ue += 1
                while ue < len(units):
                    emit_c_unit(*units[ue])
                    ue += 1

                # sb3 drain: broadcast 1/s via PE outer products into the
                # out-proj PSUM ring (GpSimd broadcasts would serialize on
                # the critical path here), normalize in place, last units.
                for h in HEAD_ORDER:
                    rbp = c_ps.tile([KC, 512], F32, tag="ps_y")
                    nc.tensor.matmul(rbp, ones_row, r1_s[3][h],
                                     start=True, stop=True)
                    ou = otu_s[3][h]
                    nc.vector.tensor_mul(ou, ou, rbp)
                    otn_s[3][h] = ou
                for u in range(NU):
                    emit_c_unit(3, u)


# ---------------------------------------------------------------------------
# Host-side input prep
# ---------------------------------------------------------------------------


def _rope_tables():
    inv_freq = 1.0 / (10000.0 ** (np.arange(0, HD, 2, dtype=np.float64) / HD))
    t = np.arange(T, dtype=np.float64)
    freqs = np.outer(t, inv_freq)                     # [T, 64]
    emb = np.concatenate([freqs, freqs], axis=-1)     # [T, 128]
    cosT = np.cos(emb).T.astype(np.float32)           # [128, T]
    sinT = np.sin(emb).T.astype(np.float32)
    # rotate_half(x) = [-x2, x1]; the device computes qrot = [x2, x1], so
    # fold the sign of the first half into the sin table.
    sinT[:HD // 2, :] *= -1.0
    return (np.ascontiguousarray(cosT.astype(BF16NP)),
            np.ascontiguousarray(sinT.astype(BF16NP)))


def _masks_t():
    # masks[r, c] = 1 if c >= r (the 128x128 triangular corner; a diagonal
    # chunk's columns beyond its first 128 are always unmasked)
    r = np.arange(KC)[:, None]
    c = np.arange(KC)[None, :]
    return (c >= r).astype(BF16NP)


def kernel(x, w_qkv, b_qkv, w_out, b_out):
    global LAST_EXEC_NS, LAST_RESULTS

    x = np.asarray(x, dtype=np.float32)
    w_qkv = np.asarray(w_qkv, dtype=np.float32)
    b_qkv = np.asarray(b_qkv, dtype=np.float32)
    w_out = np.asarray(w_out, dtype=np.float32)
    b_out = np.asarray(b_out, dtype=np.float32)

    if "prog" not in _PROGRAM_CACHE:
        _PROGRAM_CACHE["prog"] = _build_program()
    nc = _PROGRAM_CACHE["prog"]

    cosT, sinT = _rope_tables()
    masks = _masks_t()

    xTs = [np.ascontiguousarray(x[b].T.astype(BF16NP)) for b in range(B)]
    in_maps = []
    for c in range(N_CORES):
        b = c // G
        g = c % G
        f0 = g * HPG * HD
        f1 = (g + 1) * HPG * HD
        w_loc = np.ascontiguousarray(np.concatenate(
            [w_qkv[:, base + f0 + hp * 256: base + f0 + (hp + 1) * 256]
             for hp in range(HPG // 2)
             for base in (0, DIM, 2 * DIM)], axis=1).astype(BF16NP))
        b_loc = np.concatenate(
            [b_qkv[f0:f1], b_qkv[DIM + f0:DIM + f1],
             b_qkv[2 * DIM + f0:2 * DIM + f1]])
        b_cols = np.ascontiguousarray(
            b_loc.reshape(3 * HPG, HD).T).astype(np.float32)
        w_out_loc = np.ascontiguousarray(w_out[f0:f1, :].astype(BF16NP))
        in_maps.append({
            "xT": xTs[b],
            "w_qkv_loc": w_loc,
            "b_cols": b_cols,
            "w_out_loc": w_out_loc,
            "cosT": cosT,
            "sinTs": sinT,
            "masks_t": masks,
        })

    trace = bool(os.environ.get("BASS_KERNEL_TRACE"))
    res = run_bass_kernel_spmd(nc, in_maps, list(range(N_CORES)), trace=trace)
    LAST_EXEC_NS = res.exec_time_ns
    LAST_RESULTS = res

    out = np.empty((B, T, DIM), dtype=np.float32)
    for b in range(B):
        acc = res.results[4 * b]["y_part"].astype(np.float32)
        for g in range(1, G):
            acc = acc + res.results[4 * b + g]["y_part"].astype(np.float32)
        out[b] = acc + b_out[None, :]
    return out


# revision 26
# speedup vs baseline: 1.0281x; 1.0281x over previous
"""Causal self-attention (B=2, T=2048, dim=2048, 16 heads, RoPE) on 8 trn2
NeuronCores.

Sharding: core c handles batch b = c//4 and head group g = c%4 (4 heads each,
tensor-parallel over heads). Each core computes QKV projection + RoPE +
causal attention + its partial out-projection; the host sums the 4 partial
out-proj results per batch (the "all-reduce") and stacks batches.

Design (evolved 566us -> 405us phase-split -> 368us fused):
  - All matmuls bf16; QKV/attention/out-proj fully software-pipelined so the
    PE never idles long enough for the HAM clock-gate to re-throttle (the
    phase-split 405us version lost ~45us to start/transition/drain stalls
    plus the cold-clock stretch they caused).
  - Region R0: t-slice 0 QKV, kc-outer 6-psum groups (tracks weight-chunk
    DMA arrival).  Regions R1..R3: t-slice t QKV restructured output-outer
    (one psum at a time, bufs=2) woven slot-by-slot with super-block t-1's
    attention gpairs; sb's normalize chain rides the remaining slots.
    POST: sb3's attention woven with sb0-2's out-proj units, then sb3's
    units drain behind a PE-broadcast normalize.
  - PSUM budget: R0: 8-bank QKV ring. R1-3: qkv(2) + S^T(2x2) + O(1) +
    sums(1) = 8. POST: S^T(4) + O(1) + sums(1) + out-proj(2) = 8.
  - DMA: the critical first loads (x t-slice 0 jj-blocks + w_qkv halves,
    hp1 before hp0) are byte-balanced round-robin across the Sync and
    Scalar HWDGE queues in need order (first matmul ~10us instead of
    ~27us; R0's kc cadence needs ~300GB/s of weights, beyond one queue);
    x per-jj tiles so the first matmul waits on exactly one DMA; later x
    slices spread across all three queues; w_out has its own SBUF (the
    405us version's w_out DMA was WAR-blocked behind all of phase A and
    landed at ~205us, stalling the first out-proj units).  x tiles ring
    bufs=2 (tsl2/3 reuse tsl0/1's buffers).
  - RoPE rotate-half is a partition-shifted SBUF->SBUF DMA copy (sign
    folded into the host sin table); evac intermediates bf16 (halves DVE
    and rot-DMA cost vs f32).
  - Softmax sums: DVE pair-reduces each gpair's two masked exp chunks
    (and pairs consecutive non-diagonal gpairs again) so the one-hot PE
    sum matmuls stream each key only once per 2-4 chunks into a shared
    [4, 512] PSUM tile, emitted two entries late; one [4,512] bf16
    reciprocal; per-head normalize is broadcast via GpSimd (sb0-2,
    multiplied in place into the otu tile) or PE outer product (sb3
    drain).
  - Diagonal key-chunks stream only their valid query range (causal trim).
  - QKV bias via Scalar-engine activation bias; out-bias added on host;
    y written bf16 and upcast on host.
"""

import math
import os
import sys
import types

import numpy as np
import ml_dtypes

BF16NP = ml_dtypes.bfloat16

# ---------------------------------------------------------------------------
# NTFF profile hook (missing antenv.axon_hooks in this image). Reconstructed
# so run_bass_kernel_spmd(trace=True) can measure HW exec time.
# ---------------------------------------------------------------------------
try:
    import antenv

    if "antenv.axon_hooks" not in sys.modules:
        try:
            from trn_agent_boot.trn_boot import _ntff_profile_via_ctypes

            _hook = _ntff_profile_via_ctypes("/opt/axon/libaxon_pjrt.so")
        except Exception:
            _hook = None
        _m = types.ModuleType("antenv.axon_hooks")
        _m.get_axon_ntff_profile_hook = lambda: _hook
        _m.set_axon_ntff_profile_hook = lambda h: None
        sys.modules["antenv.axon_hooks"] = _m
        antenv.axon_hooks = _m
except Exception:
    pass

import concourse.bass as bass
import concourse.tile as tile
from concourse import bacc, mybir
from concourse.bass_utils import run_bass_kernel_spmd

# Problem constants (hardcoded per the task contract).
B = 2
T = 2048
DIM = 2048
H = 16
HD = 128                  # head_dim
G = 4                     # head groups (cores per batch)
HPG = H // G              # heads per group = 4
N_CORES = 8
SCALE = 1.0 / math.sqrt(HD)

F32 = mybir.dt.float32
BF16 = mybir.dt.bfloat16

TSL = 512                 # t-slice width in the projection phase
NTSL = T // TSL           # 4
QSB = 512                 # query super-block width in the attention phase
NSB = T // QSB            # 4
KC = 128                  # key chunk (partition dim)
HEAD_ORDER = (2, 3, 0, 1)

LAST_EXEC_NS = None
LAST_RESULTS = None

_PROGRAM_CACHE = {}


def _build_program():
    nc = bacc.Bacc("TRN2", target_bir_lowering=False, debug=False,
                   num_devices=N_CORES)

    xT = nc.dram_tensor("xT", [DIM, T], BF16, kind="ExternalInput").ap()
    w_qkv = nc.dram_tensor("w_qkv_loc", [DIM, 3 * HPG * HD], BF16,
                           kind="ExternalInput").ap()
    b_cols = nc.dram_tensor("b_cols", [HD, 3 * HPG], F32,
                            kind="ExternalInput").ap()
    w_out = nc.dram_tensor("w_out_loc", [HPG * HD, DIM], BF16,
                           kind="ExternalInput").ap()
    cosT = nc.dram_tensor("cosT", [HD, T], BF16, kind="ExternalInput").ap()
    sinT = nc.dram_tensor("sinTs", [HD, T], BF16, kind="ExternalInput").ap()
    masks = nc.dram_tensor("masks_t", [KC, KC], BF16,
                           kind="ExternalInput").ap()
    y = nc.dram_tensor("y_part", [T, DIM], BF16, kind="ExternalOutput").ap()

    with tile.TileContext(nc) as tc:
        _emit(tc, nc, xT, w_qkv, b_cols, w_out, cosT, sinT, masks, y)

    nc.compile()
    return nc


def _emit(tc, nc, xT, w_qkv, b_cols_d, w_out, cosT_d, sinT_d, masks_d, y):
    from contextlib import ExitStack

    ctx = ExitStack()
    with ctx:
        ctx.enter_context(nc.allow_low_precision(
            reason="bf16 matmul operands and elementwise pipeline"))
        # ---------------- constants (live for the whole kernel) -----------
        consts = ctx.enter_context(tc.tile_pool(name="consts", bufs=1))
        bcols = consts.tile([HD, 3 * HPG], F32, tag="bcols")
        nc.gpsimd.dma_start(out=bcols, in_=b_cols_d)
        # ones4[:, h, :] is the [128, 4] one-hot stationary for head h: only
        # column h is ones, so head h's softmax-sum matmul lands in row h of
        # the shared [HPG, QSB] PSUM accumulator (other rows accumulate +0).
        ones4 = consts.tile([KC, HPG, HPG], BF16, tag="ones4")
        nc.vector.memset(ones4, 0.0)
        for h in range(HPG):
            nc.vector.memset(ones4[:, h, h:h + 1], 1.0)
        ones_row = consts.tile([1, KC], BF16, tag="ones_row")
        nc.vector.memset(ones_row, 1.0)

        # QKV results: SBUF-resident for the whole kernel, split per
        # (head, t-slice) so each attention read depends on exactly the
        # producing t-slice.
        qkv_pool = ctx.enter_context(tc.tile_pool(name="qkv", bufs=1))
        qtr = [[qkv_pool.tile([HD, TSL], BF16, tag=f"qtr{h}_{t}",
                              name=f"qtr{h}_{t}") for t in range(NTSL)]
               for h in range(HPG)]
        ktr = [[qkv_pool.tile([HD, TSL], BF16, tag=f"ktr{h}_{t}",
                              name=f"ktr{h}_{t}") for t in range(NTSL)]
               for h in range(HPG)]
        vh = [[qkv_pool.tile([KC, TSL // KC, HD], BF16, tag=f"vh{h}_{t}",
                             name=f"vh{h}_{t}") for t in range(NTSL)]
              for h in range(HPG)]

        rope = ctx.enter_context(tc.tile_pool(name="rope", bufs=1))
        cosT = rope.tile([HD, T], BF16, tag="cosT")
        sinT = rope.tile([HD, T], BF16, tag="sinT")

        xT_r = xT.rearrange("(c p) t -> p c t", p=KC)        # [128, 16, T]
        w_r = w_qkv.rearrange("(c p) f -> p c f", p=KC)      # [128, 16, 1536]
        NKCH = DIM // KC                                     # 16 k-chunks
        w_out_r = w_out.rearrange("(c p) o -> p c o", p=KC)

        # ------------------- bulk-load issue plan --------------------------
        # Sync (SP) HWDGE queue: the four jj-blocks of x t-slice 0 (the
        # first matmul waits on exactly the first of these), then the
        # latency-critical small DMAs emitted throughout (rot copies, V
        # transposes, r1 stages, y writes).
        # Scalar HWDGE queue: all 16 w_qkv k-chunk rows (full 1536-col
        # rows; R0 consumes them kc-by-kc at ~1.3us each so the queue
        # stays ahead), then x slices 1..3.
        # GpSimd SWDGE queue: bias cols, rope tables, masks, w_out.
        xs_pool = ctx.enter_context(tc.tile_pool(name="xs", bufs=2))

        def load_xs(tsl, qs):
            t0 = tsl * TSL
            tiles = []
            for jj in range(4):
                xs = xs_pool.tile([KC, 4, TSL], BF16, tag=f"xs{jj}",
                                  name=f"xs{tsl}_{jj}")
                qs[jj % len(qs)].dma_start(
                    out=xs, in_=xT_r[:, jj * 4:(jj + 1) * 4, t0:t0 + TSL])
                tiles.append(xs)
            return tiles

        xs = [None] * NTSL
        xs[0] = [xs_pool.tile([KC, 4, TSL], BF16, tag=f"xs{jj}",
                              name=f"xs0_{jj}") for jj in range(4)]

        a_w = ctx.enter_context(tc.tile_pool(name="a_w", bufs=1))
        w_all = a_w.tile([KC, NKCH, 3 * HPG * HD], BF16, tag="w_all")
        # R0 consumes the hp1 (heads 2,3) half of each w row at ~150 GB/s
        # and x jj-blocks at ~100 GB/s -- more than one HWDGE queue
        # sustains.  Issue the critical loads in need order, round-robin
        # across the Sync and Scalar queues; the hp0 halves follow (needed
        # one hp-pass later), then x t-slice 1.
        crit = [("x", 0)]
        for kc in range(NKCH):
            if kc % 4 == 0 and kc > 0:
                crit.append(("x", kc // 4))
            crit.append(("w", (kc, 1)))
        crit += [("w", (kc, 0)) for kc in range(NKCH)]
        qbytes = [0, 0]
        for kind, arg in crit:
            qi = 0 if qbytes[0] <= qbytes[1] else 1
            q = (nc.sync, nc.scalar)[qi]
            if kind == "x":
                jj = arg
                q.dma_start(out=xs[0][jj],
                            in_=xT_r[:, jj * 4:(jj + 1) * 4, 0:TSL])
                qbytes[qi] += 4 * TSL * KC * 2
            else:
                kc, hp = arg
                q.dma_start(out=w_all[:, kc, hp * 768:hp * 768 + 768],
                            in_=w_r[:, kc, hp * 768:hp * 768 + 768])
                qbytes[qi] += 768 * KC * 2
        xs[1] = load_xs(1, [nc.sync, nc.scalar])

        # GpSimd (SWDGE) carries the relaxed loads: rope tables (needed at
        # ~25us), the hp0 w halves (R0's second hp pass, from ~33us), the
        # diagonal masks, then w_out.
        nc.gpsimd.dma_start(out=cosT, in_=cosT_d)
        nc.gpsimd.dma_start(out=sinT, in_=sinT_d)
        # masks_sb[r, c] = (c >= r).  A diagonal chunk dj only masks its
        # first 128 columns (beyond that c - dj*128 >= 128 > r always), so
        # one [128, 128] triangular corner serves every dj.
        masks_sb = consts.tile([KC, KC], BF16, tag="masks")
        nc.gpsimd.dma_start(out=masks_sb, in_=masks_d)
        c_w = ctx.enter_context(tc.tile_pool(name="c_w", bufs=1))
        wo = c_w.tile([KC, HPG, DIM], BF16, tag="wo")
        for hc in range(HPG):
            nc.gpsimd.dma_start(out=wo[:, hc, :], in_=w_out_r[:, hc, :])

        # ------------------- shared evac / attention pools -----------------
        a_vb = ctx.enter_context(tc.tile_pool(name="a_vb", bufs=3))
        a_qb = ctx.enter_context(tc.tile_pool(name="a_qb", bufs=4))
        a_rot = ctx.enter_context(tc.tile_pool(name="a_rot", bufs=4))
        a_m1 = ctx.enter_context(tc.tile_pool(name="a_m1", bufs=4))
        b_pt = ctx.enter_context(tc.tile_pool(name="b_pt", bufs=2))
        b_tm = ctx.enter_context(tc.tile_pool(name="b_tm", bufs=5))
        b_otn = ctx.enter_context(tc.tile_pool(name="b_otn", bufs=1))
        b_sm = ctx.enter_context(tc.tile_pool(name="b_sm", bufs=1))
        b_rb = ctx.enter_context(tc.tile_pool(name="b_rb", bufs=2))
        c_sb = ctx.enter_context(tc.tile_pool(name="c_sb", bufs=4))

        def feat0(h, kind):
            # w_qkv_loc is host-packed head-pair-major:
            # [hp0: q(2 heads), k, v | hp1: q, k, v], 256 cols per block.
            return (h // 2) * 768 + kind * 256 + (h % 2) * HD

        # ---------------- per-head QKV evacuation (R1..R3) ----------------
        def evac_v(t, h, ps_v):
            vb = a_vb.tile([HD, TSL], BF16, tag="vb")
            nc.vector.tensor_scalar_add(
                vb, ps_v, bcols[:, 2 * HPG + h:2 * HPG + h + 1])
            nc.sync.dma_start_transpose(out=vh[h][t], in_=vb)

        def evac_qk_start(h, kind, ps):
            # psum -> bf16 with bias, then launch the rotate-half copies
            # (partition-shifted SBUF->SBUF DMA; sign folded into sin).
            qb = a_qb.tile([HD, TSL], BF16, tag="qb")
            nc.scalar.activation(
                qb, ps, mybir.ActivationFunctionType.Identity,
                bias=bcols[:, kind * HPG + h:kind * HPG + h + 1])
            half = HD // 2
            qrot = a_rot.tile([HD, TSL], BF16, tag="qrot")
            nc.sync.dma_start(out=qrot[0:half, :], in_=qb[half:HD, :])
            nc.sync.dma_start(out=qrot[half:HD, :], in_=qb[0:half, :])
            return qb, qrot

        def rope_m1(t, qb):
            t0 = t * TSL
            m1 = a_m1.tile([HD, TSL], BF16, tag="m1")
            nc.vector.tensor_mul(m1, qb, cosT[:, t0:t0 + TSL])
            return m1

        def rope_fin(t, h, kind, qrot, m1):
            # m2 in place on qrot, then add into the destination tile.
            t0 = t * TSL
            nc.vector.tensor_mul(qrot, qrot, sinT[:, t0:t0 + TSL])
            dst = qtr[h] if kind == 0 else ktr[h]
            nc.vector.tensor_add(dst[t], m1, qrot)

        # ---------------- attention pop-list builder -----------------------
        # Each pop emits ~one gpair of PE work; the QKV weave calls them at
        # regular slots so emission order (= engine queue order) interleaves.
        otu_s = [dict() for _ in range(NSB)]
        otn_s = [dict() for _ in range(NSB)]
        r1_s = [dict() for _ in range(NSB)]

        def build_attn_pops(csb, ps_st_pool, ps_o_pool, ps_sum_pool):
            nk = (csb + 1) * (QSB // KC)
            ngp = nk // 2
            st = {"sum_first": True, "q": [], "ps_sum": None, "ps_o": None,
                  "tm_prev": None}

            def flush_sum(last):
                while st["q"] and (len(st["q"]) > 2 or last):
                    fh, ftm, fc0 = st["q"].pop(0)
                    nc.tensor.matmul(
                        st["ps_sum"][:, fc0:], ones4[:, fh, :], ftm[:, fc0:],
                        start=st["sum_first"],
                        stop=(last and not st["q"]),
                    )
                    st["sum_first"] = False

            pops = []

            def mk_gpair(h, gpair):
                def emit():
                    if st["ps_sum"] is None:
                        st["ps_sum"] = ps_sum_pool.tile(
                            [HPG, QSB], F32, tag="ps_sum",
                            name=f"ps_sum{csb}")
                    if gpair == 0:
                        st["ps_o"] = ps_o_pool.tile(
                            [HD, QSB], F32, tag="ps_o",
                            name=f"ps_o{csb}_{h}")
                    ps_o = st["ps_o"]
                    k0 = 2 * gpair
                    # Diagonal chunks (dj >= 0) only attend to queries
                    # q >= dj*128: trim the streamed column range of the
                    # S/O/sum matmuls, exp, and mask to the valid part.
                    djs = [(k0 + j) - (nk - QSB // KC) for j in range(2)]
                    c0s = [max(dj, 0) * KC for dj in djs]
                    cmin = min(c0s)
                    ps_st = ps_st_pool.tile([KC, 2, QSB], F32, tag="ps_st",
                                            name=f"ps_st{csb}_{h}_{gpair}")
                    for j in range(2):
                        c0 = c0s[j]
                        kci = k0 + j
                        nc.tensor.matmul(
                            ps_st[:, j, c0:],
                            ktr[h][kci // (TSL // KC)]
                               [:, (kci % (TSL // KC)) * KC:
                                (kci % (TSL // KC) + 1) * KC],
                            qtr[h][csb][:, c0:],
                            start=True, stop=True,
                        )
                    pt = b_pt.tile([KC, 2, QSB], BF16, tag="pt",
                                   name=f"pt{csb}_{h}_{gpair}")
                    nc.scalar.activation(
                        pt[:, :, cmin:], ps_st[:, :, cmin:],
                        mybir.ActivationFunctionType.Exp, scale=SCALE)
                    for j in range(2):
                        dj = djs[j]
                        if dj >= 0:
                            c0 = c0s[j]
                            nc.vector.tensor_mul(
                                pt[:, j, c0:c0 + KC], pt[:, j, c0:c0 + KC],
                                masks_sb)
                    for j in range(2):
                        kci = k0 + j
                        nc.tensor.matmul(
                            ps_o[:, c0s[j]:],
                            vh[h][kci // (TSL // KC)]
                              [:, kci % (TSL // KC), :],
                            pt[:, j, c0s[j]:],
                            start=(kci == 0), stop=(kci == nk - 1),
                        )
                    # Pair-reduce the two (masked) chunks on DVE so the
                    # softmax-sum matmul streams each gpair once instead of
                    # twice.  Diagonal gpairs: chunk 1's [cmin, c1) region
                    # holds exp() of stale psum -- zero it first.
                    if djs[1] >= 0 and c0s[1] > cmin:
                        nc.vector.memset(pt[:, 1, cmin:c0s[1]], 0.0)
                    tm = b_tm.tile([KC, QSB], BF16, tag="tm",
                                   name=f"tm{csb}_{h}_{gpair}")
                    nc.vector.tensor_add(tm[:, cmin:], pt[:, 0, cmin:],
                                         pt[:, 1, cmin:])
                    if djs[1] < 0:
                        # non-diagonal (full-range): pair with the head's
                        # previous non-diag gpair before the PE sum
                        if st["tm_prev"] is None:
                            st["tm_prev"] = tm
                        else:
                            nc.vector.tensor_add(tm, tm, st["tm_prev"])
                            st["tm_prev"] = None
                            st["q"].append((h, tm, 0))
                    else:
                        st["q"].append((h, tm, cmin))
                    flush_sum(False)
                return emit

            def mk_end_head(h):
                def emit():
                    ou = b_otn.tile([HD, QSB], BF16, tag=f"o_{csb}_{h}",
                                    name=f"otu{csb}_{h}")
                    nc.vector.tensor_copy(ou, st["ps_o"])
                    otu_s[csb][h] = ou
                return emit

            for h in HEAD_ORDER:
                for gpair in range(ngp):
                    pops.append(mk_gpair(h, gpair))
                pops.append(mk_end_head(h))

            def fin_sums():
                flush_sum(True)
                rs = b_sm.tile([HPG, QSB], BF16, tag="rsums",
                               name=f"rsums{csb}")
                nc.vector.reciprocal(rs, st["ps_sum"])
                r1_s[csb][0] = rs[0:1, :]
                for h in HEAD_ORDER:
                    if h == 0:
                        continue
                    r1 = b_sm.tile([1, QSB], BF16, tag=f"r1_{h}",
                                   name=f"r1_{csb}_{h}")
                    nc.gpsimd.dma_start(out=r1, in_=rs[h:h + 1, :])
                    r1_s[csb][h] = r1
            pops.append(fin_sums)

            if csb < NSB - 1:
                def mk_norm(h):
                    def emit():
                        rb = b_rb.tile([KC, QSB], BF16, tag="rb",
                                       name=f"rb{csb}_{h}")
                        nc.gpsimd.partition_broadcast(
                            rb, r1_s[csb][h], channels=KC)
                        ou = otu_s[csb][h]
                        nc.vector.tensor_mul(ou, ou, rb)
                        otn_s[csb][h] = ou
                    return emit
                for h in HEAD_ORDER:
                    pops.append(mk_norm(h))
            return pops

        # =================== R0: t-slice 0 QKV (kc-outer) ==================
        # Six psum accumulators advance together so the PE tracks
        # weight-chunk DMA arrival instead of stalling on the full load.
        # Heads (2,3) first: region R1's first attention pops are head 2.
        with tc.tile_pool(name="a_ps0", bufs=8, space="PSUM") as a_ps0:
            for hp in (1, 0):
                heads = (2 * hp, 2 * hp + 1)
                outs = [(h, kind) for h in heads for kind in range(3)]
                pstiles = {}
                for (h, kind) in outs:
                    pstiles[(h, kind)] = a_ps0.tile(
                        [HD, TSL], F32, tag="ps_qkv",
                        name=f"ps0_{h}_{kind}")
                for kc in range(NKCH):
                    for (h, kind) in outs:
                        nc.tensor.matmul(
                            pstiles[(h, kind)],
                            w_all[:, kc, feat0(h, kind):
                                  feat0(h, kind) + HD],
                            xs[0][kc // 4][:, kc % 4, :],
                            start=(kc == 0), stop=(kc == NKCH - 1),
                        )
                # Staged evacuation (stage fans across its engine queue
                # before the next depends on it): V evacs -> q/k evacs ->
                # transposes -> rotations -> m1 -> m2/add.
                vbs, qbs, qrots, m1s = {}, {}, {}, {}
                for h in heads:
                    vb = a_vb.tile([HD, TSL], BF16, tag="vb")
                    nc.vector.tensor_scalar_add(
                        vb, pstiles[(h, 2)],
                        bcols[:, 2 * HPG + h:2 * HPG + h + 1])
                    vbs[h] = vb
                for h in heads:
                    for kind in (0, 1):
                        qb = a_qb.tile([HD, TSL], BF16, tag="qb")
                        if hp == 0 and kind == 1:
                            nc.vector.tensor_scalar_add(
                                qb, pstiles[(h, kind)],
                                bcols[:, kind * HPG + h:
                                      kind * HPG + h + 1])
                        else:
                            nc.scalar.activation(
                                qb, pstiles[(h, kind)],
                                mybir.ActivationFunctionType.Identity,
                                bias=bcols[:, kind * HPG + h:
                                           kind * HPG + h + 1])
                        qbs[(h, kind)] = qb
                for h in heads:
                    nc.sync.dma_start_transpose(out=vh[h][0], in_=vbs[h])
                half = HD // 2
                for h in heads:
                    for kind in (0, 1):
                        qb = qbs[(h, kind)]
                        qrot = a_rot.tile([HD, TSL], BF16, tag="qrot")
                        nc.sync.dma_start(out=qrot[0:half, :],
                                          in_=qb[half:HD, :])
                        nc.sync.dma_start(out=qrot[half:HD, :],
                                          in_=qb[0:half, :])
                        qrots[(h, kind)] = qrot
                for h in heads:
                    for kind in (0, 1):
                        m1s[(h, kind)] = rope_m1(0, qbs[(h, kind)])
                for h in heads:
                    for kind in (0, 1):
                        rope_fin(0, h, kind, qrots[(h, kind)],
                                 m1s[(h, kind)])

        xs[2] = load_xs(2, [nc.gpsimd, nc.sync, nc.scalar, nc.gpsimd])

        # ============ R1..R3: QKV (output-outer) + attention weave =========
        with (
            tc.tile_pool(name="b_ps_s", bufs=2, space="PSUM") as b_ps_s,
            tc.tile_pool(name="b_ps_o", bufs=1, space="PSUM") as b_ps_o,
            tc.tile_pool(name="b_ps_sum", bufs=1, space="PSUM") as b_ps_sum,
        ):
            with tc.tile_pool(name="a_ps", bufs=2, space="PSUM") as a_ps:
                for t in range(1, NTSL):
                    pops = build_attn_pops(t - 1, b_ps_s, b_ps_o, b_ps_sum)
                    npop = len(pops)
                    nslot = 4 * 3 * 4
                    popped = 0
                    sl = 0

                    def slot():
                        nonlocal popped, sl
                        sl += 1
                        target = (npop * sl) // nslot
                        while popped < target:
                            pops[popped]()
                            popped += 1

                    pending_rope = []
                    for h in HEAD_ORDER:
                        # v output first: its transpose DMA overlaps the
                        # q/k matmuls; rot DMAs get a full output block in
                        # flight before m2 enters the DVE queue.
                        psd = {}
                        for kind in (2, 0, 1):
                            ps = a_ps.tile([HD, TSL], F32, tag="ps_qkv",
                                           name=f"ps{t}_{h}_{kind}")
                            for kc in range(NKCH):
                                nc.tensor.matmul(
                                    ps, w_all[:, kc, feat0(h, kind):
                                              feat0(h, kind) + HD],
                                    xs[t][kc // 4][:, kc % 4, :],
                                    start=(kc == 0), stop=(kc == NKCH - 1))
                                if kc % 4 == 3 and kc != NKCH - 1:
                                    slot()
                            psd[kind] = ps
                            if kind == 2:
                                evac_v(t, h, ps)
                                for cl in pending_rope:
                                    cl()
                                pending_rope = []
                            elif kind == 0:
                                psd["qb0"], psd["qr0"] = \
                                    evac_qk_start(h, 0, ps)
                            else:
                                qb1, qr1 = evac_qk_start(h, 1, ps)
                                m10 = rope_m1(t, psd["qb0"])
                                m11 = rope_m1(t, qb1)

                                def fin(t=t, h=h, qr0=psd["qr0"],
                                        qr1=qr1, m10=m10, m11=m11):
                                    rope_fin(t, h, 0, qr0, m10)
                                    rope_fin(t, h, 1, qr1, m11)
                                pending_rope = [fin]
                            slot()
                    for cl in pending_rope:
                        cl()
                    while popped < npop:
                        pops[popped]()
                        popped += 1
                    if t == 1:
                        xs[3] = load_xs(
                            3, [nc.gpsimd, nc.sync, nc.scalar,
                                nc.gpsimd])

            # ================= POST: sb3 attention + out-proj ==============
            NOB = DIM // 512
            NU = (QSB // KC) * NOB                # 16 units per super-block
            with tc.tile_pool(name="c_ps", bufs=2, space="PSUM") as c_ps:
                def emit_c_unit(csb, u):
                    tb, ob = divmod(u, NOB)
                    tt0 = tb * KC
                    o0 = ob * 512
                    ps_y = c_ps.tile([KC, 512], F32, tag="ps_y")
                    for i, hc in enumerate(HEAD_ORDER):
                        nc.tensor.matmul(
                            ps_y, otn_s[csb][hc][:, tt0:tt0 + KC],
                            wo[:, hc, o0:o0 + 512],
                            start=(i == 0), stop=(i == HPG - 1),
                        )
                    ys = c_sb.tile([KC, 512], BF16, tag="ys")
                    r0 = csb * QSB + tt0
                    if u % 2 == 0:
                        nc.scalar.activation(
                            ys, ps_y, mybir.ActivationFunctionType.Identity)
                        nc.scalar.dma_start(
                            out=y[r0:r0 + KC, o0:o0 + 512], in_=ys)
                    else:
                        nc.vector.tensor_copy(ys, ps_y)
                        nc.sync.dma_start(
                            out=y[r0:r0 + KC, o0:o0 + 512], in_=ys)

                pops = build_attn_pops(3, b_ps_s, b_ps_o, b_ps_sum)
                units = [(csb, u) for csb in range(NSB - 1)
                         for u in range(NU)]
                # Reserve the last few units to cover the sb3 drain: they
                # keep the PE fed while the reciprocal -> r1 -> broadcast
                # round-trip of the final normalize chain is in flight.
                nweave = len(units) - 6
                ue = 0
                for i, p in enumerate(pops):
                    p()
                    utarget = (nweave * (i + 1)) // len(pops)
                    while ue < utarget:
                        emit_c_unit(*units[ue])
                        ue += 1
                while ue < len(units):
                    emit_c_unit(*units[ue])
                    ue += 1

                # sb3 drain: broadcast 1/s via PE outer products into the
                # out-proj PSUM ring (GpSimd broadcasts would serialize on
                # the critical path here), normalize in place, last units.
                for h in HEAD_ORDER:
                    rbp = c_ps.tile([KC, 512], F32, tag="ps_y")
                    nc.tensor.matmul(rbp, ones_row, r1_s[3][h],
                                     start=True, stop=True)
                    ou = otu_s[3][h]
                    nc.vector.tensor_mul(ou, ou, rbp)
                    otn_s[3][h] = ou
                for u in range(NU):
                    emit_c_unit(3, u)


# ---------------------------------------------------------------------------
# Host-side input prep
# ---------------------------------------------------------------------------


def _rope_tables():
    inv_freq = 1.0 / (10000.0 ** (np.arange(0, HD, 2, dtype=np.float64) / HD))
    t = np.arange(T, dtype=np.float64)
    freqs = np.outer(t, inv_freq)                     # [T, 64]
    emb = np.concatenate([freqs, freqs], axis=-1)     # [T, 128]
    cosT = np.cos(emb).T.astype(np.float32)           # [128, T]
    sinT = np.sin(emb).T.astype(np.float32)
    # rotate_half(x) = [-x2, x1]; the device computes qrot = [x2, x1], so
    # fold the sign of the first half into the sin table.
    sinT[:HD // 2, :] *= -1.0
    return (np.ascontiguousarray(cosT.astype(BF16NP)),
            np.ascontiguousarray(sinT.astype(BF16NP)))


def _masks_t():
    # masks[r, c] = 1 if c >= r (the 128x128 triangular corner; a diagonal
    # chunk's columns beyond its first 128 are always unmasked)
    r = np.arange(KC)[:, None]
    c = np.arange(KC)[None, :]
    return (c >= r).astype(BF16NP)


def kernel(x, w_qkv, b_qkv, w_out, b_out):
    global LAST_EXEC_NS, LAST_RESULTS

    x = np.asarray(x, dtype=np.float32)
    w_qkv = np.asarray(w_qkv, dtype=np.float32)
    b_qkv = np.asarray(b_qkv, dtype=np.float32)
    w_out = np.asarray(w_out, dtype=np.float32)
    b_out = np.asarray(b_out, dtype=np.float32)

    if "prog" not in _PROGRAM_CACHE:
        _PROGRAM_CACHE["prog"] = _build_program()
    nc = _PROGRAM_CACHE["prog"]

    cosT, sinT = _rope_tables()
    masks = _masks_t()

    xTs = [np.ascontiguousarray(x[b].T.astype(BF16NP)) for b in range(B)]
    in_maps = []
    for c in range(N_CORES):
        b = c // G
        g = c % G
        f0 = g * HPG * HD
        f1 = (g + 1) * HPG * HD
        w_loc = np.ascontiguousarray(np.concatenate(
            [w_qkv[:, base + f0 + hp * 256: base + f0 + (hp + 1) * 256]
             for hp in range(HPG // 2)
             for base in (0, DIM, 2 * DIM)], axis=1).astype(BF16NP))
        b_loc = np.concatenate(
            [b_qkv[f0:f1], b_qkv[DIM + f0:DIM + f1],
             b_qkv[2 * DIM + f0:2 * DIM + f1]])
        b_cols = np.ascontiguousarray(
            b_loc.reshape(3 * HPG, HD).T).astype(np.float32)
        w_out_loc = np.ascontiguousarray(w_out[f0:f1, :].astype(BF16NP))
        in_maps.append({
            "xT": xTs[b],
            "w_qkv_loc": w_loc,
            "b_cols": b_cols,
            "w_out_loc": w_out_loc,
            "cosT": cosT,
            "sinTs": sinT,
            "masks_t": masks,
        })

    trace = bool(os.environ.get("BASS_KERNEL_TRACE"))
    res = run_bass_kernel_spmd(nc, in_maps, list(range(N_CORES)), trace=trace)
    LAST_EXEC_NS = res.exec_time_ns
    LAST_RESULTS = res

    out = np.empty((B, T, DIM), dtype=np.float32)
    for b in range(B):
        acc = res.results[4 * b]["y_part"].astype(np.float32)
        for g in range(1, G):
            acc = acc + res.results[4 * b + g]["y_part"].astype(np.float32)
        out[b] = acc + b_out[None, :]
    return out


# revision 27
# speedup vs baseline: 1.0346x; 1.0063x over previous
"""Causal self-attention (B=2, T=2048, dim=2048, 16 heads, RoPE) on 8 trn2
NeuronCores.

Sharding: core c handles batch b = c//4 and head group g = c%4 (4 heads each,
tensor-parallel over heads). Each core computes QKV projection + RoPE +
causal attention + its partial out-projection; the host sums the 4 partial
out-proj results per batch (the "all-reduce") and stacks batches.

Design (evolved 566us -> 405us phase-split -> 368us fused):
  - All matmuls bf16; QKV/attention/out-proj fully software-pipelined so the
    PE never idles long enough for the HAM clock-gate to re-throttle (the
    phase-split 405us version lost ~45us to start/transition/drain stalls
    plus the cold-clock stretch they caused).
  - Region R0: t-slice 0 QKV, kc-outer 6-psum groups (tracks weight-chunk
    DMA arrival).  Regions R1..R3: t-slice t QKV restructured output-outer
    (one psum at a time, bufs=2) woven slot-by-slot with super-block t-1's
    attention gpairs; sb's normalize chain rides the remaining slots.
    POST: sb3's attention woven with sb0-2's out-proj units, then sb3's
    units drain behind a PE-broadcast normalize.
  - PSUM budget: R0: 8-bank QKV ring. R1-3: qkv(2) + S^T(2x2) + O(1) +
    sums(1) = 8. POST: S^T(4) + O(1) + sums(1) + out-proj(2) = 8.
  - DMA: the critical first loads (x t-slice 0 jj-blocks + w_qkv halves,
    hp1 before hp0) are byte-balanced round-robin across the Sync and
    Scalar HWDGE queues in need order (first matmul ~10us instead of
    ~27us; R0's kc cadence needs ~300GB/s of weights, beyond one queue);
    x per-jj tiles so the first matmul waits on exactly one DMA; later x
    slices spread across all three queues; w_out has its own SBUF (the
    405us version's w_out DMA was WAR-blocked behind all of phase A and
    landed at ~205us, stalling the first out-proj units).  x tiles ring
    bufs=2 (tsl2/3 reuse tsl0/1's buffers).
  - RoPE rotate-half is a partition-shifted SBUF->SBUF DMA copy (sign
    folded into the host sin table); evac intermediates bf16 (halves DVE
    and rot-DMA cost vs f32).
  - Softmax sums: DVE pair-reduces each gpair's two masked exp chunks
    (and pairs consecutive non-diagonal gpairs again) so the one-hot PE
    sum matmuls stream each key only once per 2-4 chunks into a shared
    [4, 512] PSUM tile, emitted two entries late; one [4,512] bf16
    reciprocal; per-head normalize is broadcast via GpSimd (sb0-2,
    multiplied in place into the otu tile) or PE outer product (sb3
    drain).
  - Diagonal key-chunks stream only their valid query range (causal trim).
  - QKV bias via Scalar-engine activation bias; out-bias added on host;
    y written bf16 and upcast on host.
"""

import math
import os
import sys
import types

import numpy as np
import ml_dtypes

BF16NP = ml_dtypes.bfloat16

# ---------------------------------------------------------------------------
# NTFF profile hook (missing antenv.axon_hooks in this image). Reconstructed
# so run_bass_kernel_spmd(trace=True) can measure HW exec time.
# ---------------------------------------------------------------------------
try:
    import antenv

    if "antenv.axon_hooks" not in sys.modules:
        try:
            from trn_agent_boot.trn_boot import _ntff_profile_via_ctypes

            _hook = _ntff_profile_via_ctypes("/opt/axon/libaxon_pjrt.so")
        except Exception:
            _hook = None
        _m = types.ModuleType("antenv.axon_hooks")
        _m.get_axon_ntff_profile_hook = lambda: _hook
        _m.set_axon_ntff_profile_hook = lambda h: None
        sys.modules["antenv.axon_hooks"] = _m
        antenv.axon_hooks = _m
except Exception:
    pass

import concourse.bass as bass
import concourse.tile as tile
from concourse import bacc, mybir
from concourse.bass_utils import run_bass_kernel_spmd

# Problem constants (hardcoded per the task contract).
B = 2
T = 2048
DIM = 2048
H = 16
HD = 128                  # head_dim
G = 4                     # head groups (cores per batch)
HPG = H // G              # heads per group = 4
N_CORES = 8
SCALE = 1.0 / math.sqrt(HD)

F32 = mybir.dt.float32
BF16 = mybir.dt.bfloat16

TSL = 512                 # t-slice width in the projection phase
NTSL = T // TSL           # 4
QSB = 512                 # query super-block width in the attention phase
NSB = T // QSB            # 4
KC = 128                  # key chunk (partition dim)
HEAD_ORDER = (2, 3, 0, 1)

LAST_EXEC_NS = None
LAST_RESULTS = None

_PROGRAM_CACHE = {}


def _build_program():
    nc = bacc.Bacc("TRN2", target_bir_lowering=False, debug=False,
                   num_devices=N_CORES)

    xT = nc.dram_tensor("xT", [DIM, T], BF16, kind="ExternalInput").ap()
    w_qkv = nc.dram_tensor("w_qkv_loc", [DIM, 3 * HPG * HD], BF16,
                           kind="ExternalInput").ap()
    b_cols = nc.dram_tensor("b_cols", [HD, 3 * HPG], F32,
                            kind="ExternalInput").ap()
    w_out = nc.dram_tensor("w_out_loc", [HPG * HD, DIM], BF16,
                           kind="ExternalInput").ap()
    cosT = nc.dram_tensor("cosT", [HD, T], BF16, kind="ExternalInput").ap()
    sinT = nc.dram_tensor("sinTs", [HD, T], BF16, kind="ExternalInput").ap()
    masks = nc.dram_tensor("masks_t", [KC, KC], BF16,
                           kind="ExternalInput").ap()
    y = nc.dram_tensor("y_part", [T, DIM], BF16, kind="ExternalOutput").ap()

    with tile.TileContext(nc) as tc:
        _emit(tc, nc, xT, w_qkv, b_cols, w_out, cosT, sinT, masks, y)

    nc.compile()
    return nc


def _emit(tc, nc, xT, w_qkv, b_cols_d, w_out, cosT_d, sinT_d, masks_d, y):
    from contextlib import ExitStack

    ctx = ExitStack()
    with ctx:
        ctx.enter_context(nc.allow_low_precision(
            reason="bf16 matmul operands and elementwise pipeline"))
        # ---------------- constants (live for the whole kernel) -----------
        consts = ctx.enter_context(tc.tile_pool(name="consts", bufs=1))
        bcols = consts.tile([HD, 3 * HPG], F32, tag="bcols")
        nc.gpsimd.dma_start(out=bcols, in_=b_cols_d)
        # ones4[:, h, :] is the [128, 4] one-hot stationary for head h: only
        # column h is ones, so head h's softmax-sum matmul lands in row h of
        # the shared [HPG, QSB] PSUM accumulator (other rows accumulate +0).
        ones4 = consts.tile([KC, HPG, HPG], BF16, tag="ones4")
        nc.vector.memset(ones4, 0.0)
        for h in range(HPG):
            nc.vector.memset(ones4[:, h, h:h + 1], 1.0)
        ones_row = consts.tile([1, KC], BF16, tag="ones_row")
        nc.vector.memset(ones_row, 1.0)

        # QKV results: SBUF-resident for the whole kernel, split per
        # (head, t-slice) so each attention read depends on exactly the
        # producing t-slice.
        qkv_pool = ctx.enter_context(tc.tile_pool(name="qkv", bufs=1))
        qtr = [[qkv_pool.tile([HD, TSL], BF16, tag=f"qtr{h}_{t}",
                              name=f"qtr{h}_{t}") for t in range(NTSL)]
               for h in range(HPG)]
        ktr = [[qkv_pool.tile([HD, TSL], BF16, tag=f"ktr{h}_{t}",
                              name=f"ktr{h}_{t}") for t in range(NTSL)]
               for h in range(HPG)]
        vh = [[qkv_pool.tile([KC, TSL // KC, HD], BF16, tag=f"vh{h}_{t}",
                             name=f"vh{h}_{t}") for t in range(NTSL)]
              for h in range(HPG)]

        rope = ctx.enter_context(tc.tile_pool(name="rope", bufs=1))
        cosT = rope.tile([HD, T], BF16, tag="cosT")
        sinT = rope.tile([HD, T], BF16, tag="sinT")

        xT_r = xT.rearrange("(c p) t -> p c t", p=KC)        # [128, 16, T]
        w_r = w_qkv.rearrange("(c p) f -> p c f", p=KC)      # [128, 16, 1536]
        NKCH = DIM // KC                                     # 16 k-chunks
        w_out_r = w_out.rearrange("(c p) o -> p c o", p=KC)

        # ------------------- bulk-load issue plan --------------------------
        # Sync (SP) HWDGE queue: the four jj-blocks of x t-slice 0 (the
        # first matmul waits on exactly the first of these), then the
        # latency-critical small DMAs emitted throughout (rot copies, V
        # transposes, r1 stages, y writes).
        # Scalar HWDGE queue: all 16 w_qkv k-chunk rows (full 1536-col
        # rows; R0 consumes them kc-by-kc at ~1.3us each so the queue
        # stays ahead), then x slices 1..3.
        # GpSimd SWDGE queue: bias cols, rope tables, masks, w_out.
        xs_pool = ctx.enter_context(tc.tile_pool(name="xs", bufs=2))

        def load_xs(tsl, qs):
            t0 = tsl * TSL
            tiles = []
            for jj in range(4):
                xs = xs_pool.tile([KC, 4, TSL], BF16, tag=f"xs{jj}",
                                  name=f"xs{tsl}_{jj}")
                qs[jj % len(qs)].dma_start(
                    out=xs, in_=xT_r[:, jj * 4:(jj + 1) * 4, t0:t0 + TSL])
                tiles.append(xs)
            return tiles

        xs = [None] * NTSL
        xs[0] = [xs_pool.tile([KC, 4, TSL], BF16, tag=f"xs{jj}",
                              name=f"xs0_{jj}") for jj in range(4)]

        a_w = ctx.enter_context(tc.tile_pool(name="a_w", bufs=1))
        w_all = a_w.tile([KC, NKCH, 3 * HPG * HD], BF16, tag="w_all")
        # R0 consumes the hp1 (heads 2,3) half of each w row at ~150 GB/s
        # and x jj-blocks at ~100 GB/s -- more than one HWDGE queue
        # sustains.  Issue the critical loads in need order, round-robin
        # across the Sync and Scalar queues; the hp0 halves follow (needed
        # one hp-pass later), then x t-slice 1.
        crit = []
        for kc in range(NKCH):
            if kc % 4 == 0 and kc > 0:
                crit.append(("x", kc // 4))
            crit.append(("w", (kc, 1)))
        crit += [("w", (kc, 0)) for kc in range(NKCH)]
        nc.sync.dma_start(out=xs[0][0][:, 0:1, :],
                          in_=xT_r[:, 0:1, 0:TSL])
        nc.sync.dma_start(out=xs[0][0][:, 1:4, :],
                          in_=xT_r[:, 1:4, 0:TSL])
        qbytes = [4 * TSL * KC * 2, 0]
        for kind, arg in crit:
            qi = 0 if qbytes[0] <= qbytes[1] else 1
            q = (nc.sync, nc.scalar)[qi]
            if kind == "x":
                jj = arg
                q.dma_start(out=xs[0][jj],
                            in_=xT_r[:, jj * 4:(jj + 1) * 4, 0:TSL])
                qbytes[qi] += 4 * TSL * KC * 2
            else:
                kc, hp = arg
                q.dma_start(out=w_all[:, kc, hp * 768:hp * 768 + 768],
                            in_=w_r[:, kc, hp * 768:hp * 768 + 768])
                qbytes[qi] += 768 * KC * 2
        xs[1] = load_xs(1, [nc.sync, nc.scalar])

        # GpSimd (SWDGE) carries the relaxed loads: rope tables (needed at
        # ~25us), the hp0 w halves (R0's second hp pass, from ~33us), the
        # diagonal masks, then w_out.
        nc.gpsimd.dma_start(out=cosT, in_=cosT_d)
        nc.gpsimd.dma_start(out=sinT, in_=sinT_d)
        # masks_sb[r, c] = (c >= r).  A diagonal chunk dj only masks its
        # first 128 columns (beyond that c - dj*128 >= 128 > r always), so
        # one [128, 128] triangular corner serves every dj.
        masks_sb = consts.tile([KC, KC], BF16, tag="masks")
        nc.gpsimd.dma_start(out=masks_sb, in_=masks_d)
        c_w = ctx.enter_context(tc.tile_pool(name="c_w", bufs=1))
        wo = c_w.tile([KC, HPG, DIM], BF16, tag="wo")
        for hc in range(HPG):
            nc.gpsimd.dma_start(out=wo[:, hc, :], in_=w_out_r[:, hc, :])

        # ------------------- shared evac / attention pools -----------------
        a_vb = ctx.enter_context(tc.tile_pool(name="a_vb", bufs=3))
        a_qb = ctx.enter_context(tc.tile_pool(name="a_qb", bufs=4))
        a_rot = ctx.enter_context(tc.tile_pool(name="a_rot", bufs=4))
        a_m1 = ctx.enter_context(tc.tile_pool(name="a_m1", bufs=4))
        b_pt = ctx.enter_context(tc.tile_pool(name="b_pt", bufs=2))
        b_tm = ctx.enter_context(tc.tile_pool(name="b_tm", bufs=5))
        b_otn = ctx.enter_context(tc.tile_pool(name="b_otn", bufs=1))
        b_sm = ctx.enter_context(tc.tile_pool(name="b_sm", bufs=1))
        b_rb = ctx.enter_context(tc.tile_pool(name="b_rb", bufs=2))
        c_sb = ctx.enter_context(tc.tile_pool(name="c_sb", bufs=4))

        def feat0(h, kind):
            # w_qkv_loc is host-packed head-pair-major:
            # [hp0: q(2 heads), k, v | hp1: q, k, v], 256 cols per block.
            return (h // 2) * 768 + kind * 256 + (h % 2) * HD

        # ---------------- per-head QKV evacuation (R1..R3) ----------------
        def evac_v(t, h, ps_v):
            vb = a_vb.tile([HD, TSL], BF16, tag="vb")
            nc.vector.tensor_scalar_add(
                vb, ps_v, bcols[:, 2 * HPG + h:2 * HPG + h + 1])
            nc.sync.dma_start_transpose(out=vh[h][t], in_=vb)

        def evac_qk_start(h, kind, ps):
            # psum -> bf16 with bias, then launch the rotate-half copies
            # (partition-shifted SBUF->SBUF DMA; sign folded into sin).
            qb = a_qb.tile([HD, TSL], BF16, tag="qb")
            nc.scalar.activation(
                qb, ps, mybir.ActivationFunctionType.Identity,
                bias=bcols[:, kind * HPG + h:kind * HPG + h + 1])
            half = HD // 2
            qrot = a_rot.tile([HD, TSL], BF16, tag="qrot")
            nc.sync.dma_start(out=qrot[0:half, :], in_=qb[half:HD, :])
            nc.sync.dma_start(out=qrot[half:HD, :], in_=qb[0:half, :])
            return qb, qrot

        def rope_m1(t, qb):
            t0 = t * TSL
            m1 = a_m1.tile([HD, TSL], BF16, tag="m1")
            nc.vector.tensor_mul(m1, qb, cosT[:, t0:t0 + TSL])
            return m1

        def rope_fin(t, h, kind, qrot, m1):
            # m2 in place on qrot, then add into the destination tile.
            t0 = t * TSL
            nc.vector.tensor_mul(qrot, qrot, sinT[:, t0:t0 + TSL])
            dst = qtr[h] if kind == 0 else ktr[h]
            nc.vector.tensor_add(dst[t], m1, qrot)

        # ---------------- attention pop-list builder -----------------------
        # Each pop emits ~one gpair of PE work; the QKV weave calls them at
        # regular slots so emission order (= engine queue order) interleaves.
        otu_s = [dict() for _ in range(NSB)]
        otn_s = [dict() for _ in range(NSB)]
        r1_s = [dict() for _ in range(NSB)]

        def build_attn_pops(csb, ps_st_pool, ps_o_pool, ps_sum_pool):
            nk = (csb + 1) * (QSB // KC)
            ngp = nk // 2
            st = {"sum_first": True, "q": [], "ps_sum": None, "ps_o": None,
                  "tm_prev": None}

            def flush_sum(last):
                while st["q"] and (len(st["q"]) > 2 or last):
                    fh, ftm, fc0 = st["q"].pop(0)
                    nc.tensor.matmul(
                        st["ps_sum"][:, fc0:], ones4[:, fh, :], ftm[:, fc0:],
                        start=st["sum_first"],
                        stop=(last and not st["q"]),
                    )
                    st["sum_first"] = False

            pops = []

            def mk_gpair(h, gpair):
                def emit():
                    if st["ps_sum"] is None:
                        st["ps_sum"] = ps_sum_pool.tile(
                            [HPG, QSB], F32, tag="ps_sum",
                            name=f"ps_sum{csb}")
                    if gpair == 0:
                        st["ps_o"] = ps_o_pool.tile(
                            [HD, QSB], F32, tag="ps_o",
                            name=f"ps_o{csb}_{h}")
                    ps_o = st["ps_o"]
                    k0 = 2 * gpair
                    # Diagonal chunks (dj >= 0) only attend to queries
                    # q >= dj*128: trim the streamed column range of the
                    # S/O/sum matmuls, exp, and mask to the valid part.
                    djs = [(k0 + j) - (nk - QSB // KC) for j in range(2)]
                    c0s = [max(dj, 0) * KC for dj in djs]
                    cmin = min(c0s)
                    ps_st = ps_st_pool.tile([KC, 2, QSB], F32, tag="ps_st",
                                            name=f"ps_st{csb}_{h}_{gpair}")
                    for j in range(2):
                        c0 = c0s[j]
                        kci = k0 + j
                        nc.tensor.matmul(
                            ps_st[:, j, c0:],
                            ktr[h][kci // (TSL // KC)]
                               [:, (kci % (TSL // KC)) * KC:
                                (kci % (TSL // KC) + 1) * KC],
                            qtr[h][csb][:, c0:],
                            start=True, stop=True,
                        )
                    pt = b_pt.tile([KC, 2, QSB], BF16, tag="pt",
                                   name=f"pt{csb}_{h}_{gpair}")
                    nc.scalar.activation(
                        pt[:, :, cmin:], ps_st[:, :, cmin:],
                        mybir.ActivationFunctionType.Exp, scale=SCALE)
                    for j in range(2):
                        dj = djs[j]
                        if dj >= 0:
                            c0 = c0s[j]
                            nc.vector.tensor_mul(
                                pt[:, j, c0:c0 + KC], pt[:, j, c0:c0 + KC],
                                masks_sb)
                    for j in range(2):
                        kci = k0 + j
                        nc.tensor.matmul(
                            ps_o[:, c0s[j]:],
                            vh[h][kci // (TSL // KC)]
                              [:, kci % (TSL // KC), :],
                            pt[:, j, c0s[j]:],
                            start=(kci == 0), stop=(kci == nk - 1),
                        )
                    # Pair-reduce the two (masked) chunks on DVE so the
                    # softmax-sum matmul streams each gpair once instead of
                    # twice.  Diagonal gpairs: chunk 1's [cmin, c1) region
                    # holds exp() of stale psum -- zero it first.
                    if djs[1] >= 0 and c0s[1] > cmin:
                        nc.vector.memset(pt[:, 1, cmin:c0s[1]], 0.0)
                    tm = b_tm.tile([KC, QSB], BF16, tag="tm",
                                   name=f"tm{csb}_{h}_{gpair}")
                    nc.vector.tensor_add(tm[:, cmin:], pt[:, 0, cmin:],
                                         pt[:, 1, cmin:])
                    if djs[1] < 0:
                        # non-diagonal (full-range): pair with the head's
                        # previous non-diag gpair before the PE sum
                        if st["tm_prev"] is None:
                            st["tm_prev"] = tm
                        else:
                            nc.vector.tensor_add(tm, tm, st["tm_prev"])
                            st["tm_prev"] = None
                            st["q"].append((h, tm, 0))
                    else:
                        st["q"].append((h, tm, cmin))
                    flush_sum(False)
                return emit

            def mk_end_head(h):
                def emit():
                    ou = b_otn.tile([HD, QSB], BF16, tag=f"o_{csb}_{h}",
                                    name=f"otu{csb}_{h}")
                    if csb == NSB - 1:
                        # POST: DVE is busy with unit evacs; Act frees
                        # ps_o (bufs=1) sooner for the next head
                        nc.scalar.activation(
                            ou, st["ps_o"],
                            mybir.ActivationFunctionType.Identity)
                    else:
                        nc.vector.tensor_copy(ou, st["ps_o"])
                    otu_s[csb][h] = ou
                return emit

            for h in HEAD_ORDER:
                for gpair in range(ngp):
                    pops.append(mk_gpair(h, gpair))
                pops.append(mk_end_head(h))

            def fin_sums():
                flush_sum(True)
                rs = b_sm.tile([HPG, QSB], BF16, tag="rsums",
                               name=f"rsums{csb}")
                nc.vector.reciprocal(rs, st["ps_sum"])
                r1_s[csb][0] = rs[0:1, :]
                for h in HEAD_ORDER:
                    if h == 0:
                        continue
                    r1 = b_sm.tile([1, QSB], BF16, tag=f"r1_{h}",
                                   name=f"r1_{csb}_{h}")
                    nc.gpsimd.dma_start(out=r1, in_=rs[h:h + 1, :])
                    r1_s[csb][h] = r1
            pops.append(fin_sums)

            if csb < NSB - 1:
                def mk_norm(h):
                    def emit():
                        rb = b_rb.tile([KC, QSB], BF16, tag="rb",
                                       name=f"rb{csb}_{h}")
                        nc.gpsimd.partition_broadcast(
                            rb, r1_s[csb][h], channels=KC)
                        ou = otu_s[csb][h]
                        nc.vector.tensor_mul(ou, ou, rb)
                        otn_s[csb][h] = ou
                    return emit
                for h in HEAD_ORDER:
                    pops.append(mk_norm(h))
            return pops

        # =================== R0: t-slice 0 QKV (kc-outer) ==================
        # Six psum accumulators advance together so the PE tracks
        # weight-chunk DMA arrival instead of stalling on the full load.
        # Heads (2,3) first: region R1's first attention pops are head 2.
        with tc.tile_pool(name="a_ps0", bufs=8, space="PSUM") as a_ps0:
            for hp in (1, 0):
                heads = (2 * hp, 2 * hp + 1)
                outs = [(h, kind) for h in heads for kind in range(3)]
                pstiles = {}
                for (h, kind) in outs:
                    pstiles[(h, kind)] = a_ps0.tile(
                        [HD, TSL], F32, tag="ps_qkv",
                        name=f"ps0_{h}_{kind}")
                for kc in range(NKCH):
                    for (h, kind) in outs:
                        nc.tensor.matmul(
                            pstiles[(h, kind)],
                            w_all[:, kc, feat0(h, kind):
                                  feat0(h, kind) + HD],
                            xs[0][kc // 4][:, kc % 4, :],
                            start=(kc == 0), stop=(kc == NKCH - 1),
                        )
                # Staged evacuation (stage fans across its engine queue
                # before the next depends on it): V evacs -> q/k evacs ->
                # transposes -> rotations -> m1 -> m2/add.
                vbs, qbs, qrots, m1s = {}, {}, {}, {}
                for h in heads:
                    vb = a_vb.tile([HD, TSL], BF16, tag="vb")
                    nc.vector.tensor_scalar_add(
                        vb, pstiles[(h, 2)],
                        bcols[:, 2 * HPG + h:2 * HPG + h + 1])
                    vbs[h] = vb
                for h in heads:
                    for kind in (0, 1):
                        qb = a_qb.tile([HD, TSL], BF16, tag="qb")
                        if hp == 0 and kind == 1:
                            nc.vector.tensor_scalar_add(
                                qb, pstiles[(h, kind)],
                                bcols[:, kind * HPG + h:
                                      kind * HPG + h + 1])
                        else:
                            nc.scalar.activation(
                                qb, pstiles[(h, kind)],
                                mybir.ActivationFunctionType.Identity,
                                bias=bcols[:, kind * HPG + h:
                                           kind * HPG + h + 1])
                        qbs[(h, kind)] = qb
                for h in heads:
                    nc.sync.dma_start_transpose(out=vh[h][0], in_=vbs[h])
                half = HD // 2
                for h in heads:
                    for kind in (0, 1):
                        qb = qbs[(h, kind)]
                        qrot = a_rot.tile([HD, TSL], BF16, tag="qrot")
                        nc.sync.dma_start(out=qrot[0:half, :],
                                          in_=qb[half:HD, :])
                        nc.sync.dma_start(out=qrot[half:HD, :],
                                          in_=qb[0:half, :])
                        qrots[(h, kind)] = qrot
                for h in heads:
                    for kind in (0, 1):
                        m1s[(h, kind)] = rope_m1(0, qbs[(h, kind)])
                for h in heads:
                    for kind in (0, 1):
                        rope_fin(0, h, kind, qrots[(h, kind)],
                                 m1s[(h, kind)])

        xs[2] = load_xs(2, [nc.gpsimd, nc.sync, nc.scalar, nc.gpsimd])

        # ============ R1..R3: QKV (output-outer) + attention weave =========
        with (
            tc.tile_pool(name="b_ps_s", bufs=2, space="PSUM") as b_ps_s,
            tc.tile_pool(name="b_ps_o", bufs=1, space="PSUM") as b_ps_o,
            tc.tile_pool(name="b_ps_sum", bufs=1, space="PSUM") as b_ps_sum,
        ):
            with tc.tile_pool(name="a_ps", bufs=2, space="PSUM") as a_ps:
                for t in range(1, NTSL):
                    pops = build_attn_pops(t - 1, b_ps_s, b_ps_o, b_ps_sum)
                    npop = len(pops)
                    nslot = 4 * 3 * 4
                    popped = 0
                    sl = 0

                    def slot():
                        nonlocal popped, sl
                        sl += 1
                        target = (npop * sl) // nslot
                        while popped < target:
                            pops[popped]()
                            popped += 1

                    pending_rope = []
                    for h in HEAD_ORDER:
                        # v output first: its transpose DMA overlaps the
                        # q/k matmuls; rot DMAs get a full output block in
                        # flight before m2 enters the DVE queue.
                        psd = {}
                        for kind in (2, 0, 1):
                            ps = a_ps.tile([HD, TSL], F32, tag="ps_qkv",
                                           name=f"ps{t}_{h}_{kind}")
                            for kc in range(NKCH):
                                nc.tensor.matmul(
                                    ps, w_all[:, kc, feat0(h, kind):
                                              feat0(h, kind) + HD],
                                    xs[t][kc // 4][:, kc % 4, :],
                                    start=(kc == 0), stop=(kc == NKCH - 1))
                                if kc % 4 == 3 and kc != NKCH - 1:
                                    slot()
                            psd[kind] = ps
                            if kind == 2:
                                evac_v(t, h, ps)
                                for cl in pending_rope:
                                    cl()
                                pending_rope = []
                            elif kind == 0:
                                psd["qb0"], psd["qr0"] = \
                                    evac_qk_start(h, 0, ps)
                            else:
                                qb1, qr1 = evac_qk_start(h, 1, ps)
                                m10 = rope_m1(t, psd["qb0"])
                                m11 = rope_m1(t, qb1)

                                def fin(t=t, h=h, qr0=psd["qr0"],
                                        qr1=qr1, m10=m10, m11=m11):
                                    rope_fin(t, h, 0, qr0, m10)
                                    rope_fin(t, h, 1, qr1, m11)
                                pending_rope = [fin]
                            slot()
                    for cl in pending_rope:
                        cl()
                    while popped < npop:
                        pops[popped]()
                        popped += 1
                    if t == 1:
                        xs[3] = load_xs(
                            3, [nc.gpsimd, nc.sync, nc.scalar,
                                nc.gpsimd])

            # ================= POST: sb3 attention + out-proj ==============
            NOB = DIM // 512
            NU = (QSB // KC) * NOB                # 16 units per super-block
            with tc.tile_pool(name="c_ps", bufs=2, space="PSUM") as c_ps:
                def emit_c_unit(csb, u):
                    tb, ob = divmod(u, NOB)
                    tt0 = tb * KC
                    o0 = ob * 512
                    ps_y = c_ps.tile([KC, 512], F32, tag="ps_y")
                    for i, hc in enumerate(HEAD_ORDER):
                        nc.tensor.matmul(
                            ps_y, otn_s[csb][hc][:, tt0:tt0 + KC],
                            wo[:, hc, o0:o0 + 512],
                            start=(i == 0), stop=(i == HPG - 1),
                        )
                    ys = c_sb.tile([KC, 512], BF16, tag="ys")
                    r0 = csb * QSB + tt0
                    if u % 2 == 0:
                        nc.scalar.activation(
                            ys, ps_y, mybir.ActivationFunctionType.Identity)
                        nc.scalar.dma_start(
                            out=y[r0:r0 + KC, o0:o0 + 512], in_=ys)
                    else:
                        nc.vector.tensor_copy(ys, ps_y)
                        nc.sync.dma_start(
                            out=y[r0:r0 + KC, o0:o0 + 512], in_=ys)

                pops = build_attn_pops(3, b_ps_s, b_ps_o, b_ps_sum)
                units = [(csb, u) for csb in range(NSB - 1)
                         for u in range(NU)]
                # Reserve the last few units to cover the sb3 drain: they
                # keep the PE fed while the reciprocal -> r1 -> broadcast
                # round-trip of the final normalize chain is in flight.
                nweave = len(units) - 6
                ue = 0
                for i, p in enumerate(pops):
                    p()
                    utarget = (nweave * (i + 1)) // len(pops)
                    while ue < utarget:
                        emit_c_unit(*units[ue])
                        ue += 1
                while ue < len(units):
                    emit_c_unit(*units[ue])
                    ue += 1

                # sb3 drain: broadcast 1/s via PE outer products into the
                # out-proj PSUM ring (GpSimd broadcasts would serialize on
                # the critical path here), normalize in place, last units.
                for h in HEAD_ORDER:
                    rbp = c_ps.tile([KC, 512], F32, tag="ps_y")
                    nc.tensor.matmul(rbp, ones_row, r1_s[3][h],
                                     start=True, stop=True)
                    ou = otu_s[3][h]
                    nc.vector.tensor_mul(ou, ou, rbp)
                    otn_s[3][h] = ou
                for u in range(NU):
                    emit_c_unit(3, u)


# ---------------------------------------------------------------------------
# Host-side input prep
# ---------------------------------------------------------------------------


def _rope_tables():
    inv_freq = 1.0 / (10000.0 ** (np.arange(0, HD, 2, dtype=np.float64) / HD))
    t = np.arange(T, dtype=np.float64)
    freqs = np.outer(t, inv_freq)                     # [T, 64]
    emb = np.concatenate([freqs, freqs], axis=-1)     # [T, 128]
    cosT = np.cos(emb).T.astype(np.float32)           # [128, T]
    sinT = np.sin(emb).T.astype(np.float32)
    # rotate_half(x) = [-x2, x1]; the device computes qrot = [x2, x1], so
    # fold the sign of the first half into the sin table.
    sinT[:HD // 2, :] *= -1.0
    return (np.ascontiguousarray(cosT.astype(BF16NP)),
            np.ascontiguousarray(sinT.astype(BF16NP)))


def _masks_t():
    # masks[r, c] = 1 if c >= r (the 128x128 triangular corner; a diagonal
    # chunk's columns beyond its first 128 are always unmasked)
    r = np.arange(KC)[:, None]
    c = np.arange(KC)[None, :]
    return (c >= r).astype(BF16NP)


def kernel(x, w_qkv, b_qkv, w_out, b_out):
    global LAST_EXEC_NS, LAST_RESULTS

    x = np.asarray(x, dtype=np.float32)
    w_qkv = np.asarray(w_qkv, dtype=np.float32)
    b_qkv = np.asarray(b_qkv, dtype=np.float32)
    w_out = np.asarray(w_out, dtype=np.float32)
    b_out = np.asarray(b_out, dtype=np.float32)

    if "prog" not in _PROGRAM_CACHE:
        _PROGRAM_CACHE["prog"] = _build_program()
    nc = _PROGRAM_CACHE["prog"]

    cosT, sinT = _rope_tables()
    masks = _masks_t()

    xTs = [np.ascontiguousarray(x[b].T.astype(BF16NP)) for b in range(B)]
    in_maps = []
    for c in range(N_CORES):
        b = c // G
        g = c % G
        f0 = g * HPG * HD
        f1 = (g + 1) * HPG * HD
        w_loc = np.ascontiguousarray(np.concatenate(
            [w_qkv[:, base + f0 + hp * 256: base + f0 + (hp + 1) * 256]
             for hp in range(HPG // 2)
             for base in (0, DIM, 2 * DIM)], axis=1).astype(BF16NP))
        b_loc = np.concatenate(
            [b_qkv[f0:f1], b_qkv[DIM + f0:DIM + f1],
             b_qkv[2 * DIM + f0:2 * DIM + f1]])
        b_cols = np.ascontiguousarray(
            b_loc.reshape(3 * HPG, HD).T).astype(np.float32)
        w_out_loc = np.ascontiguousarray(w_out[f0:f1, :].astype(BF16NP))
        in_maps.append({
            "xT": xTs[b],
            "w_qkv_loc": w_loc,
            "b_cols": b_cols,
            "w_out_loc": w_out_loc,
            "cosT": cosT,
            "sinTs": sinT,
            "masks_t": masks,
        })

    trace = bool(os.environ.get("BASS_KERNEL_TRACE"))
    res = run_bass_kernel_spmd(nc, in_maps, list(range(N_CORES)), trace=trace)
    LAST_EXEC_NS = res.exec_time_ns
    LAST_RESULTS = res

    out = np.empty((B, T, DIM), dtype=np.float32)
    for b in range(B):
        acc = res.results[4 * b]["y_part"].astype(np.float32)
        for g in range(1, G):
            acc = acc + res.results[4 * b + g]["y_part"].astype(np.float32)
        out[b] = acc + b_out[None, :]
    return out
